# revision 1
# baseline (speedup 1.0000x reference)
import numpy as np
from contextlib import ExitStack

import concourse.bass as bass
import concourse.mybir as mybir
import concourse.tile as tile
from concourse import bacc
from concourse import bass_utils
from concourse.masks import make_identity

T, C, H, D = 4096, 768, 12, 64
N_CORES = 8
HPG = 3
GCH = HPG * D
TQ = T // 2
NTT = T // 128
NQT = TQ // 128
NST = TQ // 512
KO = C // 128
PW = 512

F32 = mybir.dt.float32
F32R = mybir.dt.float32r
AF = mybir.ActivationFunctionType
ALU = mybir.AluOpType

_CACHE = {}
_BIG_EXP = True
_CHUNK_TR = True
_STOP_AFTER = "full"


def build_nc():
    nc = bacc.Bacc(
        "TRN2", target_bir_lowering=False, debug=False, num_devices=N_CORES
    )

    x = nc.dram_tensor("x", [T, C], F32R, kind="ExternalInput").ap()
    xq = nc.dram_tensor("xq", [TQ, C], F32R, kind="ExternalInput").ap()
    wq_d = nc.dram_tensor("wq", [C, GCH], F32R, kind="ExternalInput").ap()
    wk_d = nc.dram_tensor("wk", [C, GCH], F32R, kind="ExternalInput").ap()
    wv_d = nc.dram_tensor("wv", [C, GCH], F32R, kind="ExternalInput").ap()
    bq_d = nc.dram_tensor("bq", [GCH], F32R, kind="ExternalInput").ap()
    bk_d = nc.dram_tensor("bk", [GCH], F32R, kind="ExternalInput").ap()
    bv_d = nc.dram_tensor("bv", [GCH], F32R, kind="ExternalInput").ap()
    wo_d = nc.dram_tensor("wo", [GCH, C], F32R, kind="ExternalInput").ap()
    tm_d = nc.dram_tensor("tmask", [128, 8, 512], F32R, kind="ExternalInput").ap()
    out = nc.dram_tensor("out", [C, TQ], F32, kind="ExternalOutput").ap()

    with tile.TileContext(nc) as tc, ExitStack() as ctx:
        wpool = ctx.enter_context(tc.tile_pool(name="weights", bufs=1))
        dpool = ctx.enter_context(tc.tile_pool(name="data", bufs=1))

        wq_sb = wpool.tile([128, KO, GCH], F32R, name="wq_sb")
        wk_sb = wpool.tile([128, KO, GCH], F32R, name="wk_sb")
        wv_sb = wpool.tile([128, KO, GCH], F32R, name="wv_sb")
        for sb, dr in ((wq_sb, wq_d), (wk_sb, wk_d), (wv_sb, wv_d)):
            nc.sync.dma_start(sb[:], dr.rearrange("(ko p) n -> p ko n", p=128))
        wkv1_sb = wpool.tile([128, KO, 128], F32R, name="wkv1_sb")
        nc.sync.dma_start(
            wkv1_sb[:, :, 0:64],
            wk_d[:, 128:192].rearrange("(ko p) n -> p ko n", p=128),
        )
        nc.sync.dma_start(
            wkv1_sb[:, :, 64:128],
            wv_d[:, 128:192].rearrange("(ko p) n -> p ko n", p=128),
        )
        wo_sb = [wpool.tile([64, C], F32R, name=f"wo{h}") for h in range(HPG)]
        for h in range(HPG):
            nc.sync.dma_start(wo_sb[h][:], wo_d[h * 64 : (h + 1) * 64, :])

        def bias_tile(name, dr, lo, hi):
            t = wpool.tile([hi - lo, 1], F32R, name=name)
            nc.sync.dma_start(t[:], dr[lo:hi].rearrange("(o p) -> p o", p=hi - lo))
            return t

        bq2 = bias_tile("bq2", bq_d, 0, 128)
        bq1 = bias_tile("bq1", bq_d, 128, 192)
        bk2 = bias_tile("bk2", bk_d, 0, 128)
        bv2 = bias_tile("bv2", bv_d, 0, 128)
        bkv1 = wpool.tile([128, 1], F32R, name="bkv1")
        nc.sync.dma_start(bkv1[0:64, :], bk_d[128:192].rearrange("(o p) -> p o", p=64))
        nc.sync.dma_start(bkv1[64:128, :], bv_d[128:192].rearrange("(o p) -> p o", p=64))

        tm_sb = wpool.tile([128, 8, 512], F32R, name="tm_sb")
        nc.sync.dma_start(tm_sb[:], tm_d[:])
        ident32 = wpool.tile([128, 128], F32, name="ident32")
        make_identity(nc, ident32[:])
        ident = wpool.tile([128, 128], F32R, name="ident")
        nc.vector.tensor_copy(ident[:], ident32[:])
        ones65_32 = wpool.tile([65, 64], F32, name="ones65_32")
        nc.vector.memset(ones65_32[:], 1.0)
        ones65 = wpool.tile([65, 64], F32R, name="ones65")
        nc.vector.tensor_copy(ones65[:], ones65_32[:])
        onescol = wpool.tile([128, NTT], F32, name="onescol")
        nc.vector.memset(onescol[:], 1.0)

        qT2 = dpool.tile([128, TQ], F32R, name="qT2")
        qT1 = dpool.tile([64, TQ], F32R, name="qT1")
        kT2 = dpool.tile([128, T], F32R, name="kT2")
        kvT1 = dpool.tile([128, T], F32R, name="kvT1")
        vaug = [dpool.tile([128, NTT, 72], F32R, name=f"v{h}") for h in range(HPG)]
        attnT = [dpool.tile([64, TQ], F32R, name=f"aT{h}") for h in range(HPG)]
        for h in range(HPG):
            nc.vector.tensor_copy(vaug[h][:, :, 64], onescol[:])

        def s_lhsT(h, ksl):
            if h == 0:
                return kT2[0:64, ksl]
            if h == 1:
                return kT2[64:128, ksl]
            return kvT1[0:64, ksl]

        def s_rhs(h, qsl):
            if h == 0:
                return qT2[0:64, qsl]
            if h == 1:
                return qT2[64:128, qsl]
            return qT1[0:64, qsl]

        with (
            tc.tile_pool(name="panel", bufs=2) as panpool,
            tc.tile_pool(name="stage", bufs=2) as stpool,
            tc.tile_pool(name="vt", bufs=1) as vtpool,
            tc.tile_pool(name="ab_ps", bufs=2, space="PSUM") as abps,
            tc.tile_pool(name="ab1_ps", bufs=1, space="PSUM") as abps1,
        ):

            def do_panel(src_ap, row0, panelT):
                if not _CHUNK_TR:
                    for tt in range(PW // 128):
                        st_t = stpool.tile([128, C], F32R, tag="stage")
                        r = row0 + tt * 128
                        nc.sync.dma_start(st_t[:], src_ap[r : r + 128, :])
                        for cc in range(KO):
                            ps = abps.tile([128, 128], F32R, tag="tr")
                            nc.tensor.transpose(
                                ps[:], st_t[:, cc * 128 : (cc + 1) * 128], ident[:]
                            )
                            nc.vector.tensor_copy(
                                panelT[:, cc, tt * 128 : (tt + 1) * 128], ps[:]
                            )
                    return
                for grp in range(PW // 512):
                    st4 = stpool.tile([128, 4, C], F32R, tag="stage")
                    r = row0 + grp * 512
                    nc.sync.dma_start(
                        st4[:], src_ap[r : r + 512, :].rearrange("(j p) c -> p j c", p=128)
                    )
                    stages = [st4[:, j] for j in range(4)]
                    for cc in range(KO):
                        ps = abps.tile([128, 512], F32R, tag="tr")
                        for j in range(4):
                            nc.tensor.transpose(
                                ps[:, j * 128 : (j + 1) * 128],
                                stages[j][:, cc * 128 : (cc + 1) * 128],
                                ident[:],
                            )
                        nc.vector.tensor_copy(
                            panelT[:, cc, grp * 512 : (grp + 1) * 512], ps[:]
                        )

            def proj(panelT, w_sb, csl, bias, dest, off, m):
                for st in range(PW // 512):
                    tag = "proj" if m == 128 else "proj1"
                    pool_ = abps if m == 128 else abps1
                    ps = pool_.tile([m, 512], F32, tag=tag)
                    for ko in range(KO):
                        nc.tensor.matmul(
                            ps[:],
                            w_sb[:, ko, csl],
                            panelT[:, ko, st * 512 : (st + 1) * 512],
                            start=(ko == 0),
                            stop=(ko == KO - 1),
                        )
                    nc.vector.tensor_tensor(
                        dest[:, off + st * 512 : off + (st + 1) * 512],
                        ps[:],
                        bias[:].to_broadcast([m, 512]),
                        ALU.add,
                    )

            def emit_projs(pan, kind, p):
                if kind == "q":
                    proj(pan, wq_sb, slice(0, 128), bq2, qT2, p * PW, 128)
                    proj(pan, wq_sb, slice(128, 192), bq1, qT1, p * PW, 64)
                    return
                proj(pan, wk_sb, slice(0, 128), bk2, kT2, p * PW, 128)
                proj(pan, wkv1_sb, slice(0, 128), bkv1, kvT1, p * PW, 128)
                vT2 = vtpool.tile([128, PW], F32R, tag="vT2", name="vT2")
                proj(pan, wv_sb, slice(0, 128), bv2, vT2, 0, 128)
                for tt in range(PW // 128):
                    gt = p * (PW // 128) + tt
                    tsl = slice(tt * 128, (tt + 1) * 128)
                    gsl = slice(p * PW + tt * 128, p * PW + (tt + 1) * 128)
                    for h, (src, ssl, isl) in enumerate(
                        (
                            (vT2, slice(0, 64), slice(0, 64)),
                            (vT2, slice(64, 128), slice(64, 128)),
                            (kvT1, slice(64, 128), slice(64, 128)),
                        )
                    ):
                        ps = abps.tile([128, 64], F32R, tag="vtr")
                        insl = tsl if h < 2 else gsl
                        nc.tensor.transpose(
                            ps[:], src[ssl, insl], ident[isl, isl]
                        )
                        nc.vector.tensor_copy(vaug[h][:, gt, 0:64], ps[:])

            panels = [("q", p) for p in range(TQ // PW)] + [
                ("kv", p) for p in range(T // PW)
            ]
            prev = None
            for kind, p in panels:
                pan = panpool.tile([128, KO, PW], F32R, tag="panel")
                do_panel(xq if kind == "q" else x, p * PW, pan)
                if prev is not None:
                    emit_projs(*prev)
                prev = (pan, kind, p)
            emit_projs(*prev)

        BK = 2
        LAG = 2
        with (
            tc.tile_pool(name="pe", bufs=2 + LAG) as pepool,
            tc.tile_pool(name="rc", bufs=3) as rcpool,
            tc.tile_pool(name="s_ps", bufs=2, space="PSUM") as sps,
            tc.tile_pool(name="a_ps", bufs=2, space="PSUM") as apsp,
            tc.tile_pool(name="r_ps", bufs=1, space="PSUM") as rps,
        ):
            units = [
                (h, s)
                for h in range(HPG if _STOP_AFTER != "AB" else 0)
                for s in range(NST)
            ]

            def start_norm(h, s, a_ps):
                an65 = rcpool.tile([65, 512], F32R, tag="an65")
                nc.vector.tensor_copy(an65[:], a_ps[0:65, :])
                with nc.allow_low_precision("f32r is wire-identical to f32"):
                    nc.vector.reciprocal(an65[64:65, :], an65[64:65, :])
                return (h, s, an65)

            def finish_norm(h, s, an65):
                qsl = slice(s * 512, (s + 1) * 512)
                r_ps = rps.tile([64, 512], F32, tag="rep")
                nc.tensor.matmul(
                    r_ps[:], ones65[64:65, :], an65[64:65, :], start=True, stop=True
                )
                nc.vector.tensor_tensor(
                    attnT[h][:, qsl], an65[0:64, :], r_ps[:], ALU.mult
                )

            def emit_exp(h, s, kts, bs, pe_t):
                nc.scalar.activation(
                    pe_t[:, 0 : len(kts), :],
                    bs[:, 0 : len(kts), :],
                    AF.Exp,
                    scale=0.125,
                )

            pend_pv = []
            pend_norm = []
            batch_no = [0]

            def flush_pv(keep):
                while len(pend_pv) > keep:
                    h, s, a_ps, pe_t, kts, nkt = pend_pv.pop(0)
                    for j, kt in enumerate(kts):
                        nc.tensor.matmul(
                            a_ps[:],
                            vaug[h][:, kt, 0:65],
                            pe_t[:, j, :],
                            start=(kt == 0),
                            stop=(kt == nkt - 1),
                        )
                    if kts[-1] == nkt - 1:
                        pend_norm.append((batch_no[0] + 4, start_norm(h, s, a_ps)))

            def flush_norms(force=False):
                while pend_norm and (force or pend_norm[0][0] <= batch_no[0]):
                    _, args = pend_norm.pop(0)
                    finish_norm(*args)

            for h, s in units:
                nkt = 8 * s + 8
                flush_norms(force=True)
                a_ps = apsp.tile([65, 512], F32, tag="attn")
                qsl = slice(s * 512, (s + 1) * 512)
                for kt0 in range(0, nkt, BK):
                    kts = list(range(kt0, min(kt0 + BK, nkt)))
                    bs = sps.tile([128, BK, 512], F32, tag="s")
                    for j, kt in enumerate(kts):
                        tail = kt >= 8 * s
                        nc.tensor.matmul(
                            bs[:, j, :],
                            s_lhsT(h, slice(kt * 128, (kt + 1) * 128)),
                            s_rhs(h, qsl),
                            start=True,
                            stop=not tail,
                        )
                        if tail:
                            nc.tensor.matmul(
                                bs[:, j, :],
                                ident[:],
                                tm_sb[:, kt - 8 * s, :],
                                start=False,
                                stop=True,
                            )
                    batch_no[0] += 1
                    flush_pv(LAG)
                    flush_norms()
                    pe_t = pepool.tile([128, BK, 512], F32R, tag="pe")
                    emit_exp(h, s, kts, bs, pe_t)
                    pend_pv.append((h, s, a_ps, pe_t, kts, nkt))
            flush_pv(0)
            flush_norms(force=True)

        with (
            tc.tile_pool(name="ob", bufs=3) as opool,
            tc.tile_pool(name="d_ps", bufs=2, space="PSUM") as dps,
        ):
            for oc in range(C // 128 if _STOP_AFTER == "full" else 0):
                ocs = slice(oc * 128, (oc + 1) * 128)
                ob = opool.tile([128, TQ], F32, tag="ob")
                for ts in range(NST):
                    tsl = slice(ts * 512, (ts + 1) * 512)
                    po = dps.tile([128, 512], F32, tag="o1")
                    for h in range(HPG):
                        nc.tensor.matmul(
                            po[:],
                            wo_sb[h][:, ocs],
                            attnT[h][:, tsl],
                            start=(h == 0),
                            stop=(h == HPG - 1),
                        )
                    nc.vector.tensor_copy(ob[:, tsl], po[:])
                nc.sync.dma_start(out[ocs, :], ob[:])

    nc.compile()
    return nc


def _get_nc():
    if "nc" not in _CACHE:
        _CACHE["nc"] = build_nc()
    return _CACHE["nc"]


def make_in_maps(inputs):
    x = np.ascontiguousarray(np.asarray(inputs["x"], dtype=np.float32)).reshape(T, C)
    W_qkv = np.asarray(inputs["W_qkv"], dtype=np.float32)
    b_qkv = np.asarray(inputs["b_qkv"], dtype=np.float32)
    W_out = np.asarray(inputs["W_out"], dtype=np.float32)

    NEG = np.float32(-1e9)
    diag_add = np.where(
        np.arange(128)[None, :] >= np.arange(128)[:, None], np.float32(0), NEG
    )
    tmask = {}
    for qh in (0, 1):
        m = np.zeros((128, 8, 512), np.float32)
        for ktp in range(8):
            for cg in range(4):
                rel = 2 * cg + qh
                blk = m[:, ktp, cg * 128 : (cg + 1) * 128]
                if ktp == rel:
                    blk[:] = diag_add
                elif ktp > rel:
                    blk[:] = NEG
        tmask[qh] = m

    xr = x.reshape(NTT, 128, C)
    in_maps = []
    for c in range(N_CORES):
        g, qh = c // 2, c % 2
        sl = slice(g * GCH, (g + 1) * GCH)
        in_maps.append(
            {
                "x": x,
                "xq": np.ascontiguousarray(xr[qh::2].reshape(TQ, C)),
                "wq": np.ascontiguousarray(W_qkv[:, 0 * C + g * GCH : 0 * C + (g + 1) * GCH]),
                "wk": np.ascontiguousarray(W_qkv[:, 1 * C + g * GCH : 1 * C + (g + 1) * GCH]),
                "wv": np.ascontiguousarray(W_qkv[:, 2 * C + g * GCH : 2 * C + (g + 1) * GCH]),
                "bq": np.ascontiguousarray(b_qkv[0 * C + g * GCH : 0 * C + (g + 1) * GCH]),
                "bk": np.ascontiguousarray(b_qkv[1 * C + g * GCH : 1 * C + (g + 1) * GCH]),
                "bv": np.ascontiguousarray(b_qkv[2 * C + g * GCH : 2 * C + (g + 1) * GCH]),
                "wo": np.ascontiguousarray(W_out[sl, :]),
                "tmask": tmask[qh],
            }
        )
    return in_maps


def combine_outputs(parts, b_out):
    out = np.zeros((T, C), np.float32)
    orow = out.reshape(NTT, 128, C)
    for qh in (0, 1):
        acc = parts[qh].astype(np.float32).copy()
        for g in range(1, 4):
            acc += parts[2 * g + qh]
        orow[qh::2] = np.ascontiguousarray(acc.T).reshape(NQT, 128, C)
    out += np.asarray(b_out, dtype=np.float32)[None, :]
    return out.reshape(1, T, C)


def _run(inputs, trace=False, tmpdir=None):
    nc = _get_nc()
    in_maps = make_in_maps(inputs)
    res = bass_utils.run_bass_kernel_spmd(
        nc, in_maps, core_ids=list(range(N_CORES)), trace=trace, tmpdir=tmpdir
    )
    parts = [np.asarray(res.results[c]["out"]) for c in range(N_CORES)]
    return combine_outputs(parts, inputs["b_out"]), res


def kernel(**inputs):
    out, _ = _run(inputs)
    return out



# revision 19
# speedup vs baseline: 1.6908x; 1.6908x over previous
import numpy as np
import ml_dtypes
from contextlib import ExitStack

import concourse.bass as bass
import concourse.mybir as mybir
import concourse.tile as tile
from concourse import bacc
from concourse import bass_utils
from concourse.masks import make_identity

T, C, H, D = 4096, 768, 12, 64
N_CORES = 8
HPG = 3
GCH = HPG * D
TQ = T // 2
NTT = T // 128
NQT = TQ // 128
NST = TQ // 512
KO = C // 128
PW = 512
NPAN = T // PW

F32 = mybir.dt.float32
BF16 = mybir.dt.bfloat16
AF = mybir.ActivationFunctionType
ALU = mybir.AluOpType
BF = ml_dtypes.bfloat16

_CACHE = {}
_STOP_AFTER = "full"

import os
_DBG = os.environ.get("KDBG", "0") == "1"
_KNORM = int(os.environ.get("KNORM", "0"))


def build_nc():
    nc = bacc.Bacc(
        "TRN2", target_bir_lowering=False, debug=False, num_devices=N_CORES
    )

    x = nc.dram_tensor("x", [T, C], BF16, kind="ExternalInput").ap()
    wq_d = nc.dram_tensor("wq", [C, GCH], BF16, kind="ExternalInput").ap()
    wk_d = nc.dram_tensor("wk", [C, GCH], BF16, kind="ExternalInput").ap()
    wv_d = nc.dram_tensor("wv", [C, GCH], BF16, kind="ExternalInput").ap()
    bq_d = nc.dram_tensor("bq", [GCH], F32, kind="ExternalInput").ap()
    bk_d = nc.dram_tensor("bk", [GCH], F32, kind="ExternalInput").ap()
    bv_d = nc.dram_tensor("bv", [GCH], F32, kind="ExternalInput").ap()
    wo_d = nc.dram_tensor("wo", [GCH, C], BF16, kind="ExternalInput").ap()
    bm_d = nc.dram_tensor("bmask", [128, NTT, 128], BF16, kind="ExternalInput").ap()
    out = nc.dram_tensor("out", [C, TQ], BF16, kind="ExternalOutput").ap()

    with tile.TileContext(nc) as tc, ExitStack() as ctx:
        wpool = ctx.enter_context(tc.tile_pool(name="weights", bufs=1))
        dpool = ctx.enter_context(tc.tile_pool(name="data", bufs=1))

        wq_sb = wpool.tile([128, KO, GCH], BF16, name="wq_sb")
        wk_sb = wpool.tile([128, KO, GCH], BF16, name="wk_sb")
        wv_sb = wpool.tile([128, KO, GCH], BF16, name="wv_sb")
        for sb, dr in ((wq_sb, wq_d), (wk_sb, wk_d), (wv_sb, wv_d)):
            nc.sync.dma_start(sb[:], dr.rearrange("(ko p) n -> p ko n", p=128))
        wkv1_sb = wpool.tile([128, KO, 128], BF16, name="wkv1_sb")
        nc.sync.dma_start(
            wkv1_sb[:, :, 0:64],
            wk_d[:, 128:192].rearrange("(ko p) n -> p ko n", p=128),
        )
        nc.sync.dma_start(
            wkv1_sb[:, :, 64:128],
            wv_d[:, 128:192].rearrange("(ko p) n -> p ko n", p=128),
        )
        wo01_sb = wpool.tile([128, C], BF16, name="wo01")
        nc.sync.dma_start(wo01_sb[:], wo_d[0:128, :])
        wo2_sb = wpool.tile([64, C], BF16, name="wo2")
        nc.sync.dma_start(wo2_sb[:], wo_d[128:192, :])

        def bias_tile(name, dr, lo, hi):
            t = wpool.tile([hi - lo, 1], F32, name=name)
            nc.sync.dma_start(t[:], dr[lo:hi].rearrange("(o p) -> p o", p=hi - lo))
            return t

        bq2 = bias_tile("bq2", bq_d, 0, 128)
        bq1 = bias_tile("bq1", bq_d, 128, 192)
        bk2 = bias_tile("bk2", bk_d, 0, 128)
        bv2 = bias_tile("bv2", bv_d, 0, 128)
        bkv1 = wpool.tile([128, 1], F32, name="bkv1")
        nc.sync.dma_start(bkv1[0:64, :], bk_d[128:192].rearrange("(o p) -> p o", p=64))
        nc.sync.dma_start(bkv1[64:128, :], bv_d[128:192].rearrange("(o p) -> p o", p=64))

        bm_sb = wpool.tile([128, NTT, 128], BF16, name="bm_sb")
        nc.sync.dma_start(bm_sb[:], bm_d[:])
        ident32 = wpool.tile([128, 128], F32, name="ident32")
        make_identity(nc, ident32[:])
        ident = wpool.tile([128, 128], BF16, name="ident")
        nc.vector.tensor_copy(ident[:], ident32[:])
        ones64 = wpool.tile([1, 64], BF16, name="ones64")
        nc.vector.memset(ones64[:], 1.0)

        qT2 = dpool.tile([128, TQ], BF16, name="qT2")
        qT1 = dpool.tile([64, TQ], BF16, name="qT1")
        kT2 = dpool.tile([128, T], BF16, name="kT2")
        kvT1 = dpool.tile([128, T], BF16, name="kvT1")
        vaug = [dpool.tile([128, NTT, 72], BF16, name=f"v{h}") for h in range(HPG)]
        attnT01 = dpool.tile([128, TQ], BF16, name="attnT01")
        attnT2 = dpool.tile([64, TQ], BF16, name="attnT2")
        for h in range(HPG):
            nc.vector.memset(vaug[h][:, :, 64], 1.0)

        def s_lhsT(h, ksl):
            if h == 0:
                return kT2[0:64, ksl]
            if h == 1:
                return kT2[64:128, ksl]
            return kvT1[0:64, ksl]

        def s_rhs(h, qsl):
            if h == 0:
                return qT2[0:64, qsl]
            if h == 1:
                return qT2[64:128, qsl]
            return qT1[0:64, qsl]

        def attn_dest(h, qsl):
            if h == 0:
                return attnT01[0:64, qsl]
            if h == 1:
                return attnT01[64:128, qsl]
            return attnT2[0:64, qsl]

        with (
            tc.tile_pool(name="panel", bufs=2) as panpool,
            tc.tile_pool(name="vt", bufs=2) as vtpool,
            tc.tile_pool(name="ab_ps", bufs=2, space="PSUM") as abps,
            tc.tile_pool(name="vt_ps", bufs=2, space="PSUM") as vtps,
        ):
            def do_panel(p):
                pan = panpool.tile([128, KO, PW], BF16, tag="panel")
                r0 = p * PW
                for cc in range(KO):
                    nc.sync.dma_start_transpose(
                        pan[:, cc, :], x[r0 : r0 + PW, cc * 128 : (cc + 1) * 128]
                    )
                return pan

            def proj(pan, w_sb, csl, bias, dest, off, m):
                ps = abps.tile([128, PW], F32, tag="proj")
                for ko in range(KO):
                    nc.tensor.matmul(
                        ps[0:m, :],
                        w_sb[:, ko, csl],
                        pan[:, ko, :],
                        start=(ko == 0),
                        stop=(ko == KO - 1),
                    )
                nc.vector.tensor_tensor(
                    dest[:, off : off + PW],
                    ps[0:m, :],
                    bias[:].to_broadcast([m, PW]),
                    ALU.add,
                )

            def proj_q(pan, p):
                ps2 = abps.tile([128, PW], F32, tag="proj")
                ps1 = abps.tile([128, PW], F32, tag="proj")
                for ko in range(KO):
                    rhs = pan[:, ko].rearrange("p (a b) -> p a b", a=2)[:, :, 0:128]
                    nc.tensor.matmul(
                        ps2[:, 0:256].rearrange("p (a b) -> p a b", a=2),
                        wq_sb[:, ko, 0:128],
                        rhs,
                        start=(ko == 0),
                        stop=(ko == KO - 1),
                    )
                    nc.tensor.matmul(
                        ps1[0:64, 0:256].rearrange("p (a b) -> p a b", a=2),
                        wq_sb[:, ko, 128:192],
                        rhs,
                        start=(ko == 0),
                        stop=(ko == KO - 1),
                    )
                q0 = p * 256
                nc.vector.tensor_tensor(
                    qT2[:, q0 : q0 + 256],
                    ps2[:, 0:256],
                    bq2[:].to_broadcast([128, 256]),
                    ALU.add,
                )
                nc.vector.tensor_tensor(
                    qT1[:, q0 : q0 + 256],
                    ps1[0:64, 0:256],
                    bq1[:].to_broadcast([64, 256]),
                    ALU.add,
                )

            def emit_projs(pan, p):
                proj(pan, wk_sb, slice(0, 128), bk2, kT2, p * PW, 128)
                proj(pan, wkv1_sb, slice(0, 128), bkv1, kvT1, p * PW, 128)
                vT2 = vtpool.tile([128, PW], BF16, tag="vT2")
                proj(pan, wv_sb, slice(0, 128), bv2, vT2, 0, 128)
                proj_q(pan, p)
                for tt in range(PW // 128):
                    gt = p * (PW // 128) + tt
                    tsl = slice(tt * 128, (tt + 1) * 128)
                    gsl = slice(p * PW + tt * 128, p * PW + (tt + 1) * 128)
                    for h, (src, ssl, isl) in enumerate(
                        (
                            (vT2, slice(0, 64), slice(0, 64)),
                            (vT2, slice(64, 128), slice(64, 128)),
                            (kvT1, slice(64, 128), slice(64, 128)),
                        )
                    ):
                        ps = vtps.tile([128, 64], BF16, tag="vtr")
                        insl = tsl if h < 2 else gsl
                        nc.tensor.transpose(ps[:], src[ssl, insl], ident[isl, isl])
                        nc.vector.tensor_copy(vaug[h][:, gt, 0:64], ps[:])

            prev = None
            for p in range(NPAN):
                pan = do_panel(p)
                if prev is not None:
                    emit_projs(*prev)
                prev = (pan, p)
            emit_projs(*prev)

        BK = 3
        with (
            tc.tile_pool(name="pe", bufs=4) as pepool,
            tc.tile_pool(name="an", bufs=3) as anpool,
            tc.tile_pool(name="rb", bufs=2) as rbpool,
            tc.tile_pool(name="s_ps", bufs=2, space="PSUM") as sps,
            tc.tile_pool(name="a_ps", bufs=1, space="PSUM") as apsp,
            tc.tile_pool(name="r_ps", bufs=1, space="PSUM") as rps,
        ):
            units = [
                (h, s)
                for h in range(HPG if _STOP_AFTER != "AB" else 0)
                for s in range(NST)
            ]

            pend_pv = []
            pend_norm = []
            dbg_an = (
                dpool.tile([65, len(units), 512], F32, name="dbg_an_t")
                if _DBG else None
            )

            def flush_pv(keep):
                while len(pend_pv) > keep:
                    pend_pv.pop(0)()

            def flush_norm():
                while pend_norm:
                    pend_norm.pop(0)()

            for h, s in units:
                nkt = 8 * s + 8
                a_ps = apsp.tile([65, 512], F32, tag="attn")
                qsl = slice(s * 512, (s + 1) * 512)
                for kt0 in range(0, nkt, BK):
                    kts = list(range(kt0, min(kt0 + BK, nkt)))
                    offs = [128 * max(0, kt // 2 - 4 * s) for kt in kts]
                    bs = sps.tile([128, BK, 512], F32, tag="s")
                    for j, kt in enumerate(kts):
                        nc.tensor.matmul(
                            bs[:, j, offs[j] : 512],
                            s_lhsT(h, slice(kt * 128, (kt + 1) * 128)),
                            s_rhs(h, slice(s * 512 + offs[j], (s + 1) * 512)),
                            start=True,
                            stop=True,
                        )
                    flush_pv(1)
                    flush_norm()
                    pe_t = pepool.tile([128, BK, 512], BF16, tag="pe")
                    nc.scalar.activation(
                        pe_t[:, 0 : len(kts), offs[0] : 512],
                        bs[:, 0 : len(kts), offs[0] : 512],
                        AF.Exp,
                        scale=0.125,
                    )
                    for j, kt in enumerate(kts):
                        if kt >= 8 * s:
                            o = offs[j]
                            nc.vector.tensor_tensor(
                                pe_t[:, j, o : o + 128],
                                pe_t[:, j, o : o + 128],
                                bm_sb[:, kt, :],
                                ALU.mult,
                            )

                    def emit_pv(a_ps=a_ps, pe_t=pe_t, kts=kts, offs=offs,
                                h=h, nkt=nkt):
                        for j, kt in enumerate(kts):
                            nc.tensor.matmul(
                                a_ps[:, offs[j] : 512],
                                vaug[h][:, kt, 0:65],
                                pe_t[:, j, offs[j] : 512],
                                start=(kt == 0),
                                stop=(kt == nkt - 1),
                            )

                    pend_pv.append(emit_pv)
                flush_pv(0)

                an = anpool.tile([65, 512], F32, tag="an65")
                nc.vector.tensor_copy(an[:], a_ps[:])
                if _DBG:
                    nc.vector.tensor_copy(
                        dbg_an[:, units.index((h, s)), :], an[:]
                    )
                if _KNORM == 1:
                    def finish_raw(h=h, qsl=qsl, an=an):
                        nc.vector.tensor_copy(attn_dest(h, qsl), an[0:64, :])
                    pend_norm.append(finish_raw)
                else:
                    rc = rbpool.tile([1, 512], F32, tag="rc")
                    if _KNORM == 2:
                        with nc.allow_low_precision("recip"):
                            nc.vector.reciprocal(rc[:], an[64:65, :])
                    else:
                        rden = rbpool.tile([1, 512], F32, tag="rden")
                        nc.sync.dma_start(rden[:], an[64:65, :])
                        nc.vector.reciprocal_approx_fast(
                            out=rc[:], in_=rden[:]
                        )
                    rcb = rbpool.tile([1, 512], BF16, tag="rcb")
                    nc.vector.tensor_copy(rcb[:], rc[:])

                    def finish_norm(h=h, qsl=qsl, an=an, rcb=rcb):
                        r_ps = rps.tile([64, 512], F32, tag="rep")
                        nc.tensor.matmul(
                            r_ps[:], ones64[:], rcb[:], start=True, stop=True
                        )
                        nc.vector.tensor_tensor(
                            attn_dest(h, qsl), an[0:64, :], r_ps[:], ALU.mult
                        )

                    pend_norm.append(finish_norm)
            flush_norm()

        if _DBG:
            da = nc.dram_tensor("dbg_an", [65, 12 * 512], F32,
                                kind="ExternalOutput").ap()
            nc.sync.dma_start(da[:], dbg_an[:].rearrange("p a b -> p (a b)"))
            for nm, src, rows in (
                ("dbg_qT2", qT2, 128), ("dbg_qT1", qT1, 64),
                ("dbg_kT2", kT2, 128), ("dbg_kvT1", kvT1, 128),
                ("dbg_aT01", attnT01, 128), ("dbg_aT2", attnT2, 64),
            ):
                cols = src.shape[1]
                d = nc.dram_tensor(nm, [rows, cols], BF16, kind="ExternalOutput").ap()
                nc.sync.dma_start(d[:], src[:])
            dv = nc.dram_tensor("dbg_v0", [128, NTT * 72], BF16,
                                kind="ExternalOutput").ap()
            nc.sync.dma_start(
                dv[:], vaug[0][:].rearrange("p a b -> p (a b)")
            )

        with (
            tc.tile_pool(name="ob", bufs=2) as opool,
            tc.tile_pool(name="d_ps", bufs=2, space="PSUM") as dps,
        ):
            for oc in range(C // 128 if _STOP_AFTER == "full" else 0):
                ocs = slice(oc * 128, (oc + 1) * 128)
                ob = opool.tile([128, TQ], BF16, tag="ob")
                for ts in range(NST):
                    tsl = slice(ts * 512, (ts + 1) * 512)
                    po = dps.tile([128, 512], F32, tag="o1")
                    nc.tensor.matmul(
                        po[:], wo01_sb[:, ocs], attnT01[:, tsl],
                        start=True, stop=False,
                    )
                    nc.tensor.matmul(
                        po[:], wo2_sb[:, ocs], attnT2[:, tsl],
                        start=False, stop=True,
                    )
                    nc.vector.tensor_copy(ob[:, tsl], po[:])
                nc.sync.dma_start(out[ocs, :], ob[:])

    nc.compile()
    return nc


def _get_nc():
    if "nc" not in _CACHE:
        _CACHE["nc"] = build_nc()
    return _CACHE["nc"]


def make_in_maps(inputs):
    x = np.ascontiguousarray(np.asarray(inputs["x"], dtype=np.float32)).reshape(T, C)
    W_qkv = np.asarray(inputs["W_qkv"], dtype=np.float32)
    b_qkv = np.asarray(inputs["b_qkv"], dtype=np.float32)
    W_out = np.asarray(inputs["W_out"], dtype=np.float32)

    xb = x.astype(BF)
    perm = np.arange(NTT) ^ 1
    x_by_parity = {
        0: xb,
        1: np.ascontiguousarray(xb.reshape(NTT, 128, C)[perm].reshape(T, C)),
    }

    tri = np.triu(np.ones((128, 128), np.float32)).astype(BF)
    bmask = {}
    for qh in (0, 1):
        m = np.zeros((128, NTT, 128), np.float32)
        for kt in range(NTT):
            if kt % 2 == 0:
                m[:, kt, :] = tri
            else:
                m[:, kt, :] = 0.0 if qh == 0 else 1.0
        bmask[qh] = m.astype(BF)

    in_maps = []
    for c in range(N_CORES):
        g, qh = c // 2, c % 2
        sl = slice(g * GCH, (g + 1) * GCH)
        in_maps.append(
            {
                "x": x_by_parity[qh],
                "wq": np.ascontiguousarray(W_qkv[:, 0 * C + g * GCH : 0 * C + (g + 1) * GCH]).astype(BF),
                "wk": np.ascontiguousarray(W_qkv[:, 1 * C + g * GCH : 1 * C + (g + 1) * GCH]).astype(BF),
                "wv": np.ascontiguousarray(W_qkv[:, 2 * C + g * GCH : 2 * C + (g + 1) * GCH]).astype(BF),
                "bq": np.ascontiguousarray(b_qkv[0 * C + g * GCH : 0 * C + (g + 1) * GCH]),
                "bk": np.ascontiguousarray(b_qkv[1 * C + g * GCH : 1 * C + (g + 1) * GCH]),
                "bv": np.ascontiguousarray(b_qkv[2 * C + g * GCH : 2 * C + (g + 1) * GCH]),
                "wo": np.ascontiguousarray(W_out[sl, :]).astype(BF),
                "bmask": bmask[qh],
            }
        )
    return in_maps


def combine_outputs(parts, b_out):
    out = np.zeros((T, C), np.float32)
    orow = out.reshape(NTT, 128, C)
    for qh in (0, 1):
        acc = parts[qh].astype(np.float32)
        for g in range(1, 4):
            acc = acc + parts[2 * g + qh].astype(np.float32)
        orow[qh::2] = np.ascontiguousarray(acc.T).reshape(NQT, 128, C)
    out += np.asarray(b_out, dtype=np.float32)[None, :]
    return out.reshape(1, T, C)


def _run(inputs, trace=False, tmpdir=None):
    nc = _get_nc()
    in_maps = make_in_maps(inputs)
    res = bass_utils.run_bass_kernel_spmd(
        nc, in_maps, core_ids=list(range(N_CORES)), trace=trace, tmpdir=tmpdir
    )
    parts = [np.asarray(res.results[c]["out"]) for c in range(N_CORES)]
    return combine_outputs(parts, inputs["b_out"]), res


def kernel(**inputs):
    out, _ = _run(inputs)
    return out


# revision 22
# speedup vs baseline: 1.7585x; 1.0400x over previous
import numpy as np
import ml_dtypes
from contextlib import ExitStack

import concourse.bass as bass
import concourse.mybir as mybir
import concourse.tile as tile
from concourse import bacc
from concourse import bass_utils
from concourse.masks import make_identity

T, C, H, D = 4096, 768, 12, 64
N_CORES = 8
HPG = 3
GCH = HPG * D
TQ = T // 2
NTT = T // 128
NQT = TQ // 128
NST = TQ // 512
KO = C // 128
PW = 512
NPAN = T // PW

F32 = mybir.dt.float32
BF16 = mybir.dt.bfloat16
AF = mybir.ActivationFunctionType
ALU = mybir.AluOpType
BF = ml_dtypes.bfloat16

_CACHE = {}
_STOP_AFTER = "full"

import os
_DBG = os.environ.get("KDBG", "0") == "1"
_KNORM = int(os.environ.get("KNORM", "0"))


def build_nc():
    nc = bacc.Bacc(
        "TRN2", target_bir_lowering=False, debug=False, num_devices=N_CORES
    )

    x = nc.dram_tensor("x", [T, C], BF16, kind="ExternalInput").ap()
    wq_d = nc.dram_tensor("wq", [C, GCH], BF16, kind="ExternalInput").ap()
    wk_d = nc.dram_tensor("wk", [C, GCH], BF16, kind="ExternalInput").ap()
    wv_d = nc.dram_tensor("wv", [C, GCH], BF16, kind="ExternalInput").ap()
    bq_d = nc.dram_tensor("bq", [GCH], F32, kind="ExternalInput").ap()
    bk_d = nc.dram_tensor("bk", [GCH], F32, kind="ExternalInput").ap()
    bv_d = nc.dram_tensor("bv", [GCH], F32, kind="ExternalInput").ap()
    wo_d = nc.dram_tensor("wo", [GCH, C], BF16, kind="ExternalInput").ap()
    bm_d = nc.dram_tensor("bmask", [128, NTT, 128], BF16, kind="ExternalInput").ap()
    out = nc.dram_tensor("out", [C, TQ], BF16, kind="ExternalOutput").ap()

    with tile.TileContext(nc) as tc, ExitStack() as ctx:
        wpool = ctx.enter_context(tc.tile_pool(name="weights", bufs=1))
        dpool = ctx.enter_context(tc.tile_pool(name="data", bufs=1))

        wq_sb = wpool.tile([128, KO, GCH], BF16, name="wq_sb")
        wk_sb = wpool.tile([128, KO, GCH], BF16, name="wk_sb")
        wv_sb = wpool.tile([128, KO, GCH], BF16, name="wv_sb")
        for sb, dr in ((wq_sb, wq_d), (wk_sb, wk_d), (wv_sb, wv_d)):
            nc.sync.dma_start(sb[:], dr.rearrange("(ko p) n -> p ko n", p=128))
        wkv1_sb = wpool.tile([128, KO, 128], BF16, name="wkv1_sb")
        nc.sync.dma_start(
            wkv1_sb[:, :, 0:64],
            wk_d[:, 128:192].rearrange("(ko p) n -> p ko n", p=128),
        )
        nc.sync.dma_start(
            wkv1_sb[:, :, 64:128],
            wv_d[:, 128:192].rearrange("(ko p) n -> p ko n", p=128),
        )
        wo01_sb = wpool.tile([128, C], BF16, name="wo01")
        nc.sync.dma_start(wo01_sb[:], wo_d[0:128, :])
        wo2_sb = wpool.tile([64, C], BF16, name="wo2")
        nc.sync.dma_start(wo2_sb[:], wo_d[128:192, :])

        def bias_tile(name, dr, lo, hi):
            t = wpool.tile([hi - lo, 1], F32, name=name)
            nc.sync.dma_start(t[:], dr[lo:hi].rearrange("(o p) -> p o", p=hi - lo))
            return t

        bq2 = bias_tile("bq2", bq_d, 0, 128)
        bq1 = bias_tile("bq1", bq_d, 128, 192)
        bk2 = bias_tile("bk2", bk_d, 0, 128)
        bv2 = bias_tile("bv2", bv_d, 0, 128)
        bkv1 = wpool.tile([128, 1], F32, name="bkv1")
        nc.sync.dma_start(bkv1[0:64, :], bk_d[128:192].rearrange("(o p) -> p o", p=64))
        nc.sync.dma_start(bkv1[64:128, :], bv_d[128:192].rearrange("(o p) -> p o", p=64))

        bm_sb = wpool.tile([128, NTT, 128], BF16, name="bm_sb")
        nc.sync.dma_start(bm_sb[:], bm_d[:])
        ident32 = wpool.tile([128, 128], F32, name="ident32")
        make_identity(nc, ident32[:])
        ident = wpool.tile([128, 128], BF16, name="ident")
        nc.vector.tensor_copy(ident[:], ident32[:])
        ones64 = wpool.tile([1, 64], BF16, name="ones64")
        nc.vector.memset(ones64[:], 1.0)

        qT2 = dpool.tile([128, TQ], BF16, name="qT2")
        qT1 = dpool.tile([64, TQ], BF16, name="qT1")
        kT2 = dpool.tile([128, T], BF16, name="kT2")
        kvT1 = dpool.tile([128, T], BF16, name="kvT1")
        vaug = [dpool.tile([128, NTT, 72], BF16, name=f"v{h}") for h in range(HPG)]
        attnT01 = dpool.tile([128, TQ], BF16, name="attnT01")
        attnT2 = dpool.tile([64, TQ], BF16, name="attnT2")
        for h in range(HPG):
            nc.vector.memset(vaug[h][:, :, 64], 1.0)

        def s_lhsT(h, ksl):
            if h == 0:
                return kT2[0:64, ksl]
            if h == 1:
                return kT2[64:128, ksl]
            return kvT1[0:64, ksl]

        def s_rhs(h, qsl):
            if h == 0:
                return qT2[0:64, qsl]
            if h == 1:
                return qT2[64:128, qsl]
            return qT1[0:64, qsl]

        def attn_dest(h, qsl):
            if h == 0:
                return attnT01[0:64, qsl]
            if h == 1:
                return attnT01[64:128, qsl]
            return attnT2[0:64, qsl]

        with (
            tc.tile_pool(name="panel", bufs=2) as panpool,
            tc.tile_pool(name="vt", bufs=2) as vtpool,
            tc.tile_pool(name="ab_ps", bufs=2, space="PSUM") as abps,
            tc.tile_pool(name="vt_ps", bufs=2, space="PSUM") as vtps,
        ):
            def do_panel(p):
                pan = panpool.tile([128, KO, PW], BF16, tag="panel")
                r0 = p * PW
                for cc in range(KO):
                    nc.sync.dma_start_transpose(
                        pan[:, cc, :], x[r0 : r0 + PW, cc * 128 : (cc + 1) * 128]
                    )
                return pan

            def proj(pan, w_sb, csl, bias, dest, off, m):
                ps = abps.tile([128, PW], F32, tag="proj")
                for ko in range(KO):
                    nc.tensor.matmul(
                        ps[0:m, :],
                        w_sb[:, ko, csl],
                        pan[:, ko, :],
                        start=(ko == 0),
                        stop=(ko == KO - 1),
                    )
                nc.vector.tensor_tensor(
                    dest[:, off : off + PW],
                    ps[0:m, :],
                    bias[:].to_broadcast([m, PW]),
                    ALU.add,
                )

            def proj_q(pan, p):
                ps2 = abps.tile([128, PW], F32, tag="proj")
                ps1 = abps.tile([128, PW], F32, tag="proj")
                for ko in range(KO):
                    rhs = pan[:, ko].rearrange("p (a b) -> p a b", a=2)[:, :, 0:128]
                    nc.tensor.matmul(
                        ps2[:, 0:256].rearrange("p (a b) -> p a b", a=2),
                        wq_sb[:, ko, 0:128],
                        rhs,
                        start=(ko == 0),
                        stop=(ko == KO - 1),
                    )
                    nc.tensor.matmul(
                        ps1[0:64, 0:256].rearrange("p (a b) -> p a b", a=2),
                        wq_sb[:, ko, 128:192],
                        rhs,
                        start=(ko == 0),
                        stop=(ko == KO - 1),
                    )
                q0 = p * 256
                nc.vector.tensor_tensor(
                    qT2[:, q0 : q0 + 256],
                    ps2[:, 0:256],
                    bq2[:].to_broadcast([128, 256]),
                    ALU.add,
                )
                nc.vector.tensor_tensor(
                    qT1[:, q0 : q0 + 256],
                    ps1[0:64, 0:256],
                    bq1[:].to_broadcast([64, 256]),
                    ALU.add,
                )

            def emit_projs(pan, p):
                proj(pan, wk_sb, slice(0, 128), bk2, kT2, p * PW, 128)
                proj(pan, wkv1_sb, slice(0, 128), bkv1, kvT1, p * PW, 128)
                vT2 = vtpool.tile([128, PW], BF16, tag="vT2")
                proj(pan, wv_sb, slice(0, 128), bv2, vT2, 0, 128)
                proj_q(pan, p)
                for tt in range(PW // 128):
                    gt = p * (PW // 128) + tt
                    tsl = slice(tt * 128, (tt + 1) * 128)
                    gsl = slice(p * PW + tt * 128, p * PW + (tt + 1) * 128)
                    for h, (src, ssl, isl) in enumerate(
                        (
                            (vT2, slice(0, 64), slice(0, 64)),
                            (vT2, slice(64, 128), slice(64, 128)),
                            (kvT1, slice(64, 128), slice(64, 128)),
                        )
                    ):
                        ps = vtps.tile([128, 64], BF16, tag="vtr")
                        insl = tsl if h < 2 else gsl
                        nc.tensor.transpose(ps[:], src[ssl, insl], ident[isl, isl])
                        nc.vector.tensor_copy(vaug[h][:, gt, 0:64], ps[:])

            prev = None
            for p in range(NPAN):
                pan = do_panel(p)
                if prev is not None:
                    emit_projs(*prev)
                prev = (pan, p)
            emit_projs(*prev)

        BK = 3
        with (
            tc.tile_pool(name="pe", bufs=4) as pepool,
            tc.tile_pool(name="an", bufs=3) as anpool,
            tc.tile_pool(name="rb", bufs=2) as rbpool,
            tc.tile_pool(name="s_ps", bufs=2, space="PSUM") as sps,
            tc.tile_pool(name="a_ps", bufs=1, space="PSUM") as apsp,
            tc.tile_pool(name="r_ps", bufs=1, space="PSUM") as rps,
        ):
            units = [
                (h, s)
                for h in range(HPG if _STOP_AFTER != "AB" else 0)
                for s in range(NST)
            ]

            pend_pv = []
            pend_norm = []
            dbg_an = (
                dpool.tile([65, len(units), 512], F32, name="dbg_an_t")
                if _DBG else None
            )

            def flush_pv(keep):
                while len(pend_pv) > keep:
                    pend_pv.pop(0)()

            def flush_norm():
                while pend_norm:
                    pend_norm.pop(0)()

            for h, s in units:
                nkt = 8 * s + 8
                a_ps = apsp.tile([65, 512], F32, tag="attn")
                qsl = slice(s * 512, (s + 1) * 512)
                off_of = lambda kt: 128 * max(0, kt // 2 - 4 * s)
                groups = {}
                for kt in range(nkt):
                    groups.setdefault(off_of(kt), []).append(kt)
                batches = []
                for off_v in sorted(groups):
                    g = groups[off_v]
                    for i in range(0, len(g), BK):
                        batches.append(g[i : i + BK])
                for kts in batches:
                    offs = [off_of(kt) for kt in kts]
                    bs = sps.tile([128, BK, 512], F32, tag="s")
                    for j, kt in enumerate(kts):
                        nc.tensor.matmul(
                            bs[:, j, offs[j] : 512],
                            s_lhsT(h, slice(kt * 128, (kt + 1) * 128)),
                            s_rhs(h, slice(s * 512 + offs[j], (s + 1) * 512)),
                            start=True,
                            stop=True,
                        )
                    flush_pv(1)
                    flush_norm()
                    pe_t = pepool.tile([128, BK, 512], BF16, tag="pe")
                    nc.scalar.activation(
                        pe_t[:, 0 : len(kts), offs[0] : 512],
                        bs[:, 0 : len(kts), offs[0] : 512],
                        AF.Exp,
                        scale=0.125,
                    )
                    for j, kt in enumerate(kts):
                        if kt >= 8 * s:
                            o = offs[j]
                            nc.vector.tensor_tensor(
                                pe_t[:, j, o : o + 128],
                                pe_t[:, j, o : o + 128],
                                bm_sb[:, kt, :],
                                ALU.mult,
                            )

                    def emit_pv(a_ps=a_ps, pe_t=pe_t, kts=kts, offs=offs,
                                h=h, nkt=nkt):
                        for j, kt in enumerate(kts):
                            nc.tensor.matmul(
                                a_ps[:, offs[j] : 512],
                                vaug[h][:, kt, 0:65],
                                pe_t[:, j, offs[j] : 512],
                                start=(kt == 0),
                                stop=(kt == nkt - 1),
                            )

                    pend_pv.append(emit_pv)
                flush_pv(0)

                an = anpool.tile([65, 512], F32, tag="an65")
                nc.vector.tensor_copy(an[:], a_ps[:])
                if _DBG:
                    nc.vector.tensor_copy(
                        dbg_an[:, units.index((h, s)), :], an[:]
                    )
                if _KNORM == 1:
                    def finish_raw(h=h, qsl=qsl, an=an):
                        nc.vector.tensor_copy(attn_dest(h, qsl), an[0:64, :])
                    pend_norm.append(finish_raw)
                else:
                    rc = rbpool.tile([1, 512], F32, tag="rc")
                    if _KNORM == 2:
                        with nc.allow_low_precision("recip"):
                            nc.vector.reciprocal(rc[:], an[64:65, :])
                    else:
                        rden = rbpool.tile([1, 512], F32, tag="rden")
                        nc.sync.dma_start(rden[:], an[64:65, :])
                        nc.vector.reciprocal_approx_fast(
                            out=rc[:], in_=rden[:]
                        )
                    rcb = rbpool.tile([1, 512], BF16, tag="rcb")
                    nc.vector.tensor_copy(rcb[:], rc[:])

                    def finish_norm(h=h, qsl=qsl, an=an, rcb=rcb):
                        r_ps = rps.tile([64, 512], F32, tag="rep")
                        nc.tensor.matmul(
                            r_ps[:], ones64[:], rcb[:], start=True, stop=True
                        )
                        nc.vector.tensor_tensor(
                            attn_dest(h, qsl), an[0:64, :], r_ps[:], ALU.mult
                        )

                    pend_norm.append(finish_norm)
            flush_norm()

        if _DBG:
            da = nc.dram_tensor("dbg_an", [65, 12 * 512], F32,
                                kind="ExternalOutput").ap()
            nc.sync.dma_start(da[:], dbg_an[:].rearrange("p a b -> p (a b)"))
            for nm, src, rows in (
                ("dbg_qT2", qT2, 128), ("dbg_qT1", qT1, 64),
                ("dbg_kT2", kT2, 128), ("dbg_kvT1", kvT1, 128),
                ("dbg_aT01", attnT01, 128), ("dbg_aT2", attnT2, 64),
            ):
                cols = src.shape[1]
                d = nc.dram_tensor(nm, [rows, cols], BF16, kind="ExternalOutput").ap()
                nc.sync.dma_start(d[:], src[:])
            dv = nc.dram_tensor("dbg_v0", [128, NTT * 72], BF16,
                                kind="ExternalOutput").ap()
            nc.sync.dma_start(
                dv[:], vaug[0][:].rearrange("p a b -> p (a b)")
            )

        with (
            tc.tile_pool(name="ob", bufs=2) as opool,
            tc.tile_pool(name="d_ps", bufs=2, space="PSUM") as dps,
        ):
            for oc in range(C // 128 if _STOP_AFTER == "full" else 0):
                ocs = slice(oc * 128, (oc + 1) * 128)
                ob = opool.tile([128, TQ], BF16, tag="ob")
                for ts in range(NST):
                    tsl = slice(ts * 512, (ts + 1) * 512)
                    po = dps.tile([128, 512], F32, tag="o1")
                    nc.tensor.matmul(
                        po[:], wo01_sb[:, ocs], attnT01[:, tsl],
                        start=True, stop=False,
                    )
                    nc.tensor.matmul(
                        po[:], wo2_sb[:, ocs], attnT2[:, tsl],
                        start=False, stop=True,
                    )
                    nc.vector.tensor_copy(ob[:, tsl], po[:])
                nc.sync.dma_start(out[ocs, :], ob[:])

    nc.compile()
    return nc


def _get_nc():
    if "nc" not in _CACHE:
        _CACHE["nc"] = build_nc()
    return _CACHE["nc"]


def make_in_maps(inputs):
    x = np.ascontiguousarray(np.asarray(inputs["x"], dtype=np.float32)).reshape(T, C)
    W_qkv = np.asarray(inputs["W_qkv"], dtype=np.float32)
    b_qkv = np.asarray(inputs["b_qkv"], dtype=np.float32)
    W_out = np.asarray(inputs["W_out"], dtype=np.float32)

    xb = x.astype(BF)
    perm = np.arange(NTT) ^ 1
    x_by_parity = {
        0: xb,
        1: np.ascontiguousarray(xb.reshape(NTT, 128, C)[perm].reshape(T, C)),
    }

    tri = np.triu(np.ones((128, 128), np.float32)).astype(BF)
    bmask = {}
    for qh in (0, 1):
        m = np.zeros((128, NTT, 128), np.float32)
        for kt in range(NTT):
            if kt % 2 == 0:
                m[:, kt, :] = tri
            else:
                m[:, kt, :] = 0.0 if qh == 0 else 1.0
        bmask[qh] = m.astype(BF)

    in_maps = []
    for c in range(N_CORES):
        g, qh = c // 2, c % 2
        sl = slice(g * GCH, (g + 1) * GCH)
        in_maps.append(
            {
                "x": x_by_parity[qh],
                "wq": np.ascontiguousarray(W_qkv[:, 0 * C + g * GCH : 0 * C + (g + 1) * GCH]).astype(BF),
                "wk": np.ascontiguousarray(W_qkv[:, 1 * C + g * GCH : 1 * C + (g + 1) * GCH]).astype(BF),
                "wv": np.ascontiguousarray(W_qkv[:, 2 * C + g * GCH : 2 * C + (g + 1) * GCH]).astype(BF),
                "bq": np.ascontiguousarray(b_qkv[0 * C + g * GCH : 0 * C + (g + 1) * GCH]),
                "bk": np.ascontiguousarray(b_qkv[1 * C + g * GCH : 1 * C + (g + 1) * GCH]),
                "bv": np.ascontiguousarray(b_qkv[2 * C + g * GCH : 2 * C + (g + 1) * GCH]),
                "wo": np.ascontiguousarray(W_out[sl, :]).astype(BF),
                "bmask": bmask[qh],
            }
        )
    return in_maps


def combine_outputs(parts, b_out):
    out = np.zeros((T, C), np.float32)
    orow = out.reshape(NTT, 128, C)
    for qh in (0, 1):
        acc = parts[qh].astype(np.float32)
        for g in range(1, 4):
            acc = acc + parts[2 * g + qh].astype(np.float32)
        orow[qh::2] = np.ascontiguousarray(acc.T).reshape(NQT, 128, C)
    out += np.asarray(b_out, dtype=np.float32)[None, :]
    return out.reshape(1, T, C)


def _run(inputs, trace=False, tmpdir=None):
    nc = _get_nc()
    in_maps = make_in_maps(inputs)
    res = bass_utils.run_bass_kernel_spmd(
        nc, in_maps, core_ids=list(range(N_CORES)), trace=trace, tmpdir=tmpdir
    )
    parts = [np.asarray(res.results[c]["out"]) for c in range(N_CORES)]
    return combine_outputs(parts, inputs["b_out"]), res


def kernel(**inputs):
    out, _ = _run(inputs)
    return out
